# revision 2
# baseline (speedup 1.0000x reference)
import numpy as np

SR, SEG, NH, BASE_F = 48000, 960, 8, 220.0
N, C, Lf = 32, 256, 250
Lw = Lf * SEG
NCORES = 8
NPC = N // NCORES   # 4 samples per core
HP = Lf // 2        # 125 segments per half
MAGIC = 12582912.0  # 1.5*2^23 (exactly representable in bf16)
CB = 8              # coarse mag block (samples per constant-mag block)
NCB = SEG // CB     # 120 coarse blocks per segment

_cache = {}


def _bf16(x):
    import ml_dtypes
    return np.asarray(x, np.float32).astype(ml_dtypes.bfloat16)


def _consts():
    import ml_dtypes
    s = np.arange(SEG, dtype=np.float64)
    delta = (s + 0.5) / SEG - 0.5
    lo = s < SEG // 2
    a_s = np.where(lo, -delta, 0.0)
    b_s = np.where(lo, 1 + delta, 1 - delta)
    d_s = np.where(lo, 0.0, delta)
    A = np.cumsum(a_s) / SR
    B = np.cumsum(b_s) / SR
    D = np.cumsum(d_s) / SR
    Cm = np.stack([A, B, D]).astype(np.float32)          # [3, 960]
    C_hi = Cm.astype(ml_dtypes.bfloat16)
    C_lo = (Cm - C_hi.astype(np.float32)).astype(ml_dtypes.bfloat16)
    one = np.ones(SEG, np.float32)
    # V rows: [gtaps(9), bhi, blo, MAGIC, MAGIC, gtaps(9), bhi, blo]
    # rt rows: [-C..., -1, -1, +1, -1, +C..., +1, +1] => psum = u - round(u)
    crows = []
    for t in range(3):
        crows += [C_hi[t].astype(np.float32), C_lo[t].astype(np.float32),
                  C_hi[t].astype(np.float32)]
    neg = [-r for r in crows] + [-one, -one]
    mid = [one, -one]
    pos = crows + [one, one]
    rtu = np.stack(neg + mid + pos).astype(ml_dtypes.bfloat16)  # [24, 960]
    # mag interp weights at coarse block centers (piecewise-constant blocks)
    jj = np.arange(NCB) * CB + (CB - 1) / 2.0
    j0 = np.floor(jj).astype(int)
    w = (jj - j0).astype(np.float64)
    rmc = np.stack([
        (1 - w) * np.stack([a_s, b_s, d_s])[:, j0][i] + w * np.stack([a_s, b_s, d_s])[:, np.minimum(j0 + 1, SEG - 1)][i]
        for i in range(3)
    ]).astype(ml_dtypes.bfloat16)                        # [3, 120]
    m8 = np.full((NH, 1), 1.0 / NH, ml_dtypes.bfloat16)
    return rtu, rmc, m8


def _build():
    import concourse.bacc as bacc
    import concourse.mybir as mybir
    import concourse.tile as tile
    from contextlib import ExitStack

    f32 = mybir.dt.float32
    bf = mybir.dt.bfloat16
    AF = mybir.ActivationFunctionType
    AL = mybir.AluOpType
    LN2 = float(np.log(2.0))
    TWO_PI = float(2.0 * np.pi)
    c1, c2 = 120.0 / SR, 720.0 / SR

    nc = bacc.Bacc("TRN2", target_bir_lowering=False, debug=False)
    x_d = nc.dram_tensor("x", [NPC, C, Lf], f32, kind="ExternalInput")
    phi_d = nc.dram_tensor("phi", [NPC, 1], f32, kind="ExternalInput")
    wh_d = nc.dram_tensor("wh", [C, 40], bf, kind="ExternalInput")
    wl_d = nc.dram_tensor("wl", [C, 40], bf, kind="ExternalInput")
    bm_d = nc.dram_tensor("bmag", [NH, 1], f32, kind="ExternalInput")
    bo_d = nc.dram_tensor("boct", [1, 1], f32, kind="ExternalInput")
    rtu_d = nc.dram_tensor("rtu", [24, SEG], bf, kind="ExternalInput")
    rmc_d = nc.dram_tensor("rmc", [3, NCB], bf, kind="ExternalInput")
    m8_d = nc.dram_tensor("m8", [NH, 1], bf, kind="ExternalInput")
    out_d = nc.dram_tensor("wave", [NPC, Lf, SEG], bf, kind="ExternalOutput")
    scr_g = nc.dram_tensor("scr_g", [NPC, 756], bf, kind="Internal")


    with tile.TileContext(nc) as tc, ExitStack() as ctx:
        const = ctx.enter_context(tc.tile_pool(name="const", bufs=1))
        xp = ctx.enter_context(tc.tile_pool(name="xp", bufs=1))
        frame = ctx.enter_context(tc.tile_pool(name="frame", bufs=1))
        vw = ctx.enter_context(tc.tile_pool(name="vw", bufs=1))
        vt_p = ctx.enter_context(tc.tile_pool(name="vt_p", bufs=2))
        sn_p = ctx.enter_context(tc.tile_pool(name="sn_p", bufs=2))
        wv_p = ctx.enter_context(tc.tile_pool(name="wv_p", bufs=3))
        # PSUM: four 2-bank pools, tiles rotate through them
        pA = ctx.enter_context(tc.tile_pool(name="pA", bufs=1, space="PSUM"))
        pB = ctx.enter_context(tc.tile_pool(name="pB", bufs=1, space="PSUM"))
        pC = ctx.enter_context(tc.tile_pool(name="pC", bufs=1, space="PSUM"))
        pD = ctx.enter_context(tc.tile_pool(name="pD", bufs=1, space="PSUM"))

        # ---- weights first (scalar q), x by halves on both queues ----
        wha = const.tile([128, 40], bf)
        nc.scalar.dma_start(wha[:], wh_d[0:128, :])
        whb = const.tile([128, 40], bf)
        nc.scalar.dma_start(whb[:], wh_d[128:256, :])
        wla = const.tile([128, 40], bf)
        nc.scalar.dma_start(wla[:], wl_d[0:128, :])
        wlb = const.tile([128, 40], bf)
        nc.scalar.dma_start(wlb[:], wl_d[128:256, :])
        xv = x_d[:].rearrange("n c l -> c n l")
        xa = xp.tile([128, NPC * Lf], f32)
        nc.sync.dma_start(xa[:, 0:500].rearrange("c (n l) -> c n l", n=2),
                          xv[0:128, 0:2, :])
        nc.sync.dma_start(xa[:, 500:1000].rearrange("c (n l) -> c n l", n=2),
                          xv[0:128, 2:4, :])
        xb = xp.tile([128, NPC * Lf], f32)
        nc.sync.dma_start(xb[:, 0:500].rearrange("c (n l) -> c n l", n=2),
                          xv[128:256, 0:2, :])
        nc.sync.dma_start(xb[:, 500:1000].rearrange("c (n l) -> c n l", n=2),
                          xv[128:256, 2:4, :])
        xha = xp.tile([128, NPC * Lf], bf)
        xla = xp.tile([128, NPC * Lf], bf)
        xhb = xp.tile([128, NPC * Lf], bf)
        xlb = xp.tile([128, NPC * Lf], bf)
        for lo_, hi_ in [(0, 500), (500, 1000)]:
            nc.vector.tensor_copy(xha[:, lo_:hi_], xa[:, lo_:hi_])
            nc.vector.tensor_tensor(xla[:, lo_:hi_], xa[:, lo_:hi_], xha[:, lo_:hi_],
                                    AL.subtract)
            nc.vector.tensor_copy(xhb[:, lo_:hi_], xb[:, lo_:hi_])
            nc.vector.tensor_tensor(xlb[:, lo_:hi_], xb[:, lo_:hi_], xhb[:, lo_:hi_],
                                    AL.subtract)
        bo = const.tile([1, 1], f32)
        nc.scalar.dma_start(bo[:], bo_d[:])
        bm = const.tile([NH, 1], f32)
        nc.scalar.dma_start(bm[:], bm_d[:])
        rtu = const.tile([24, SEG], bf)
        nc.sync.dma_start(rtu[:], rtu_d[:])
        rmc = const.tile([3, NCB], bf)
        nc.sync.dma_start(rmc[:], rmc_d[:])
        m8 = const.tile([NH, 1], bf)
        nc.sync.dma_start(m8[:], m8_d[:])
        phi = const.tile([NPC, 1], f32)
        nc.sync.dma_start(phi[:], phi_d[:])

        # ---- PE clock warmup while x loads (results unused) ----
        wtile = const.tile([12, 512], bf)
        nc.vector.memset(wtile[:], 0.0)
        pwarm = pD.tile([128, 1024], f32, tag="s")
        for _ in range(6):
            nc.tensor.matmul(pwarm[:, 0:480], wtile[:, 0:128], wtile[:, 0:480],
                             start=True, stop=True)

        # ---- conv via bf16 hi/lo split: pc40 [40, 1000] ----
        NT = NPC * Lf  # 1000
        pc40 = pA.tile([40, 1024], f32, tag="s")
        for half, (c0, c1_) in enumerate([(0, 500), (500, 1000)]):
            pcol = half * 512
            out = pc40[:, pcol:pcol + 500]
            nc.tensor.matmul(out, wha[:], xha[:, c0:c1_], start=True, stop=False)
            nc.tensor.matmul(out, whb[:], xhb[:, c0:c1_], start=False, stop=False)
            nc.tensor.matmul(out, wla[:], xha[:, c0:c1_], start=False, stop=False)
            nc.tensor.matmul(out, wlb[:], xhb[:, c0:c1_], start=False, stop=False)
            nc.tensor.matmul(out, wha[:], xla[:, c0:c1_], start=False, stop=False)
            nc.tensor.matmul(out, whb[:], xlb[:, c0:c1_], start=False, stop=True)
        pcv = pc40[:].rearrange("p (b c) -> p b c", b=2)[:, :, 0:500]
        gall = frame.tile([1, NT], f32)
        nc.scalar.activation(gall[:].rearrange("p (b c) -> p b c", b=2),
                             pcv[32:33], AF.Exp, bias=bo[0:1, 0:1], scale=LN2)
        magbf = frame.tile([NH, NT], bf)
        nc.scalar.activation(magbf[:].rearrange("p (b c) -> p b c", b=2),
                             pcv[0:NH], AF.Exp, bias=bm[:, 0:1])

        # ---- g row -> gtp4 [4, 252] padded (pads via Pool copies) ----
        gtp4 = frame.tile([NPC, Lf + 2], f32)
        nc.sync.dma_start(gtp4[:, 1:Lf + 1],
                          gall[:].rearrange("p (n l) -> p n l", n=NPC))
        nc.vector.tensor_copy(gtp4[:, 0:1], gtp4[:, 1:2])
        nc.vector.tensor_copy(gtp4[:, Lf + 1:Lf + 2], gtp4[:, Lf:Lf + 1])

        # ---- g splits staged for DRAM hop: ghilo [4, 756] = [ghi, ghi, glo] ----
        ghilo = frame.tile([NPC, 756], bf)
        nc.gpsimd.tensor_copy(ghilo[:, 0:252], gtp4[:])
        nc.gpsimd.tensor_copy(ghilo[:, 252:504], gtp4[:])
        nc.gpsimd.tensor_tensor(ghilo[:, 504:756], gtp4[:], ghilo[:, 0:252], AL.subtract)
        nc.sync.dma_start(scr_g[:], ghilo[:])

        # ---- segment sums + scan (Pool, [4, 250]) ----
        gsum = frame.tile([NPC, Lf], f32)
        nc.vector.tensor_tensor(gsum[:], gtp4[:, 0:Lf], gtp4[:, 2:Lf + 2], AL.add)
        t2 = frame.tile([NPC, Lf], f32)
        nc.vector.tensor_scalar(t2[:], gtp4[:, 1:Lf + 1], c2, None, AL.mult)
        sst = frame.tile([NPC, Lf], f32)
        nc.vector.scalar_tensor_tensor(sst[:], gsum[:], c1, t2[:], AL.mult, AL.add)
        rS = frame.tile([NPC, Lf], f32)
        nc.vector.tensor_scalar(rS[:], sst[:], MAGIC, MAGIC, AL.add, AL.subtract)
        sf = frame.tile([NPC, Lf], f32)
        nc.vector.tensor_tensor(sf[:], sst[:], rS[:], AL.subtract)
        pinc = frame.tile([NPC, Lf], f32)
        nc.vector.tensor_tensor_scan(pinc[:], sf[:], sf[:], 0.0, AL.add, AL.bypass)
        base = frame.tile([NPC, Lf], f32)
        nc.vector.scalar_tensor_tensor(base[:], pinc[:], phi[:, 0:1], sf[:],
                                       AL.add, AL.subtract)
        rB = frame.tile([NPC, Lf], f32)
        nc.vector.tensor_scalar(rB[:], base[:], MAGIC, MAGIC, AL.add, AL.subtract)
        nc.vector.tensor_tensor(base[:], base[:], rB[:], AL.subtract)

        # ---- base splits staged: basell [4, 750] = [bhi, blo, MAGIC] ----
        basell = frame.tile([NPC, 500], bf)
        mrow8 = frame.tile([8, Lf], bf)
        nc.vector.memset(mrow8[:], MAGIC)
        nc.vector.tensor_copy(basell[:, 0:250], base[:])
        nc.vector.tensor_tensor(basell[:, 250:500], base[:], basell[:, 0:250],
                                AL.subtract)


        # ---- mbar = mean over harmonics (copies on Act) ----
        pmb = pB.tile([1, 1024], f32, tag="s")
        nc.tensor.matmul(pmb[:, 0:500], m8[:], magbf[:, 0:500], start=True, stop=True)
        nc.tensor.matmul(pmb[:, 512:1012], m8[:], magbf[:, 500:1000], start=True, stop=True)
        mb_all = frame.tile([1, NPC * (Lf + 2)], bf)   # padded per-sample [1, 4*252]
        mbv = mb_all[:].rearrange("p (n l) -> p n l", n=NPC)
        nc.scalar.activation(mbv[:, 0:2, 1:Lf + 1], pmb[:, 0:500].rearrange(
            "p (n l) -> p n l", n=2), AF.Copy)
        nc.scalar.activation(mbv[:, 2:4, 1:Lf + 1], pmb[:, 512:1012].rearrange(
            "p (n l) -> p n l", n=2), AF.Copy)
        nc.scalar.activation(mbv[:, :, 0:1], mbv[:, :, 1:2], AF.Copy)
        nc.scalar.activation(mbv[:, :, Lf + 1:Lf + 2], mbv[:, :, Lf:Lf + 1], AF.Copy)

        # ---- V12 [12, 1000] bf16, cols = (sample, segment) ----
        # rows 0-8: (copy c, tap tau) = 3c+tau; c0/c1 = ghi, c2 = glo
        # rows 9-11: bhi, blo, MAGIC
        V12 = vw.tile([24, NPC * Lf], bf)
        gview = scr_g[:].rearrange("n (c w) -> c n w", c=3)
        for tau, eng in [(0, nc.sync), (1, nc.scalar), (2, nc.gpsimd)]:
            eng.dma_start(
                V12[3 * tau:3 * tau + 3, :].rearrange("r (n l) -> r n l", n=NPC),
                gview[:, :, tau:tau + Lf])
        for tau, eng in [(0, nc.sync), (1, nc.scalar), (2, nc.gpsimd)]:
            eng.dma_start(
                V12[13 + 3 * tau:13 + 3 * tau + 3, :].rearrange("r (n l) -> r n l", n=NPC),
                gview[:, :, tau:tau + Lf])
        for r, row, eng in [(0, 9, nc.sync), (1, 10, nc.scalar), (0, 22, nc.gpsimd),
                            (1, 23, nc.sync)]:
            eng.dma_start(
                V12[row:row + 1, :].rearrange("p (n l) -> p n l", n=NPC),
                basell[:, 250 * r:250 * r + 250].rearrange("n (p l) -> n p l", p=1))
        nc.scalar.dma_start(V12[11:12, :].rearrange("p (n l) -> p n l", n=NPC),
                            mrow8[0:4, :].rearrange("n (p l) -> n p l", p=1))
        nc.gpsimd.dma_start(V12[12:13, :].rearrange("p (n l) -> p n l", n=NPC),
                            mrow8[4:8, :].rearrange("n (p l) -> n p l", p=1))

        W12 = vw.tile([3, NPC * Lf], bf)
        for tau, eng in [(0, nc.sync), (1, nc.gpsimd), (2, nc.scalar)]:
            eng.dma_start(W12[tau:tau + 1, :].rearrange("p (n l) -> p n l", n=NPC),
                          mbv[:, :, tau:tau + Lf])
        # ---- mag interp (coarse) for all 8 chunks into one psum tile ----
        pm = pC.tile([HP, 1024], f32, tag="s")
        for q in range(8):
            n_, h = q // 2, q % 2
            col = (q % 4) * NCB + (q // 4) * 512
            nc.tensor.matmul(pm[:, col:col + NCB],
                             W12[:, n_ * Lf + h * HP:n_ * Lf + (h + 1) * HP],
                             rmc[:], start=True, stop=True)
        mvs = frame.tile([HP, 8 * NCB], f32)
        nc.vector.tensor_copy(mvs[:].rearrange("p (b c) -> p b c", b=2, c=4 * NCB),
                              pm[:].rearrange("p (b c) -> p b c", b=2)[:, :, 0:4 * NCB])

        # ---- chunk loop ----
        upools = [pA, pB, pC, pD]
        waves = {}
        for q in range(8):
            n_, h = q // 2, q % 2
            cols = slice(n_ * Lf + h * HP, n_ * Lf + (h + 1) * HP)
            pv = upools[q % 4].tile([HP, 1024], f32, tag="s")
            nc.tensor.matmul(pv[:, 0:480], V12[:, cols],
                             rtu[:, 0:480], start=True, stop=True)
            nc.tensor.matmul(pv[:, 512:992], V12[:, cols],
                             rtu[:, 480:960], start=True, stop=True)
            # psum already holds v = u - round(u); sn = sin(2pi*v) = sin(2pi*u)
            snb = sn_p.tile([HP, SEG], bf)
            nc.scalar.activation(snb[:].rearrange("p (b c) -> p b c", b=2),
                                 pv[:].rearrange("p (b c) -> p b c", b=2)[:, :, 0:480],
                                 AF.Sin, scale=TWO_PI * (1.0 - 2e-6))
            # wave = sn * mag  (Pool; mag piecewise-constant over CB samples)
            wave = wv_p.tile([HP, SEG], bf)
            mcol = (q % 4) * NCB + (q // 4) * 4 * NCB
            nc.gpsimd.tensor_tensor(
                wave[:].rearrange("p (i j) -> p i j", j=CB),
                snb[:].rearrange("p (i j) -> p i j", j=CB),
                mvs[:, mcol:mcol + NCB].rearrange("p (i j) -> p i j", j=1).broadcast_to(
                    (HP, NCB, CB)),
                AL.mult)
            nc.sync.dma_start(out_d[n_, h * HP:(h + 1) * HP, :], wave[:])

    nc.compile()
    return nc


def kernel(x, phi, w_mag, b_mag, w_oct, b_oct):
    from concourse.bass_utils import run_bass_kernel_spmd
    import ml_dtypes

    if "nc" not in _cache:
        _cache["nc"] = _build()
    nc = _cache["nc"]

    import ml_dtypes
    rtu, rmc, m8 = _consts()
    wT40 = np.zeros((C, 40), np.float32)
    wT40[:, 0:NH] = w_mag[:, :, 0].T
    wT40[:, 32] = w_oct[0, :, 0]
    wh = wT40.astype(ml_dtypes.bfloat16)
    wl = (wT40 - wh.astype(np.float32)).astype(ml_dtypes.bfloat16)
    bm = np.asarray(b_mag, np.float32).reshape(NH, 1)
    bo = np.array([[np.log(220.0) + np.log(2.0) * float(np.asarray(b_oct).ravel()[0])]],
                  np.float32)
    in_maps = []
    for c in range(NCORES):
        in_maps.append(dict(
            x=np.ascontiguousarray(x[c * NPC:(c + 1) * NPC]).astype(np.float32),
            phi=np.ascontiguousarray(phi[c * NPC:(c + 1) * NPC, 0]).astype(np.float32),
            wh=wh, wl=wl, bmag=bm, boct=bo, rtu=rtu, rmc=rmc, m8=m8,
        ))
    res = run_bass_kernel_spmd(nc, in_maps, core_ids=list(range(NCORES)))
    waves = [res.results[c]["wave"].astype(np.float32).reshape(NPC, 1, Lw)
             for c in range(NCORES)]
    return np.concatenate(waves, axis=0)


# revision 5
# speedup vs baseline: 1.0030x; 1.0030x over previous
import numpy as np

SR, SEG, NH, BASE_F = 48000, 960, 8, 220.0
N, C, Lf = 32, 256, 250
Lw = Lf * SEG
NCORES = 8
NPC = N // NCORES   # 4 samples per core
HP = Lf // 2        # 125 segments per half
MAGIC = 12582912.0  # 1.5*2^23 (exactly representable in bf16)
CB = 8              # coarse mag block (samples per constant-mag block)
NCB = SEG // CB     # 120 coarse blocks per segment

_cache = {}


def _bf16(x):
    import ml_dtypes
    return np.asarray(x, np.float32).astype(ml_dtypes.bfloat16)


def _consts():
    import ml_dtypes
    s = np.arange(SEG, dtype=np.float64)
    delta = (s + 0.5) / SEG - 0.5
    lo = s < SEG // 2
    a_s = np.where(lo, -delta, 0.0)
    b_s = np.where(lo, 1 + delta, 1 - delta)
    d_s = np.where(lo, 0.0, delta)
    A = np.cumsum(a_s) / SR
    B = np.cumsum(b_s) / SR
    D = np.cumsum(d_s) / SR
    Cm = np.stack([A, B, D]).astype(np.float32)          # [3, 960]
    C_hi = Cm.astype(ml_dtypes.bfloat16)
    C_lo = (Cm - C_hi.astype(np.float32)).astype(ml_dtypes.bfloat16)
    one = np.ones(SEG, np.float32)
    # V rows: [gtaps(9), bhi, blo, MAGIC, MAGIC, gtaps(9), bhi, blo]
    # rt rows: [-C..., -1, -1, +1, -1, +C..., +1, +1] => psum = u - round(u)
    crows = []
    for t in range(3):
        crows += [C_hi[t].astype(np.float32), C_lo[t].astype(np.float32),
                  C_hi[t].astype(np.float32)]
    neg = [-r for r in crows] + [-one, -one]
    mid = [one, -one]
    pos = crows + [one, one]
    rtu = np.stack(neg + mid + pos).astype(ml_dtypes.bfloat16)  # [24, 960]
    # mag interp weights at coarse block centers (piecewise-constant blocks)
    jj = np.arange(NCB) * CB + (CB - 1) / 2.0
    j0 = np.floor(jj).astype(int)
    w = (jj - j0).astype(np.float64)
    rmc = np.stack([
        (1 - w) * np.stack([a_s, b_s, d_s])[:, j0][i] + w * np.stack([a_s, b_s, d_s])[:, np.minimum(j0 + 1, SEG - 1)][i]
        for i in range(3)
    ]).astype(ml_dtypes.bfloat16)                        # [3, 120]
    m8 = np.full((NH, 1), 1.0 / NH, ml_dtypes.bfloat16)
    return rtu, rmc, m8


def _build():
    import concourse.bacc as bacc
    import concourse.mybir as mybir
    import concourse.tile as tile
    from contextlib import ExitStack

    f32 = mybir.dt.float32
    bf = mybir.dt.bfloat16
    AF = mybir.ActivationFunctionType
    AL = mybir.AluOpType
    LN2 = float(np.log(2.0))
    TWO_PI = float(2.0 * np.pi)
    c1, c2 = 120.0 / SR, 720.0 / SR

    nc = bacc.Bacc("TRN2", target_bir_lowering=False, debug=False)
    x_d = nc.dram_tensor("x", [NPC, C, Lf], f32, kind="ExternalInput")
    phi_d = nc.dram_tensor("phi", [NPC, 1], f32, kind="ExternalInput")
    wh_d = nc.dram_tensor("wh", [C, 40], bf, kind="ExternalInput")
    wl_d = nc.dram_tensor("wl", [C, 40], bf, kind="ExternalInput")
    bm_d = nc.dram_tensor("bmag", [NH, 1], f32, kind="ExternalInput")
    bo_d = nc.dram_tensor("boct", [1, 1], f32, kind="ExternalInput")
    rtu_d = nc.dram_tensor("rtu", [24, SEG], bf, kind="ExternalInput")
    rmc_d = nc.dram_tensor("rmc", [3, NCB], bf, kind="ExternalInput")
    m8_d = nc.dram_tensor("m8", [NH, 1], bf, kind="ExternalInput")
    out_d = nc.dram_tensor("wave", [NPC, Lf, SEG], bf, kind="ExternalOutput")
    scr_g = nc.dram_tensor("scr_g", [NPC, 756], bf, kind="Internal")


    with tile.TileContext(nc) as tc, ExitStack() as ctx:
        const = ctx.enter_context(tc.tile_pool(name="const", bufs=1))
        xp = ctx.enter_context(tc.tile_pool(name="xp", bufs=1))
        frame = ctx.enter_context(tc.tile_pool(name="frame", bufs=1))
        vw = ctx.enter_context(tc.tile_pool(name="vw", bufs=1))
        vt_p = ctx.enter_context(tc.tile_pool(name="vt_p", bufs=2))
        sn_p = ctx.enter_context(tc.tile_pool(name="sn_p", bufs=2))
        wv_p = ctx.enter_context(tc.tile_pool(name="wv_p", bufs=3))
        # PSUM: four 2-bank pools, tiles rotate through them
        pA = ctx.enter_context(tc.tile_pool(name="pA", bufs=1, space="PSUM"))
        pB = ctx.enter_context(tc.tile_pool(name="pB", bufs=1, space="PSUM"))
        pC = ctx.enter_context(tc.tile_pool(name="pC", bufs=1, space="PSUM"))
        pD = ctx.enter_context(tc.tile_pool(name="pD", bufs=1, space="PSUM"))

        # ---- weights first (scalar q), x by halves on both queues ----
        wha = const.tile([128, 40], bf)
        nc.scalar.dma_start(wha[:], wh_d[0:128, :])
        whb = const.tile([128, 40], bf)
        nc.scalar.dma_start(whb[:], wh_d[128:256, :])
        wla = const.tile([128, 40], bf)
        nc.scalar.dma_start(wla[:], wl_d[0:128, :])
        wlb = const.tile([128, 40], bf)
        nc.scalar.dma_start(wlb[:], wl_d[128:256, :])
        xv = x_d[:].rearrange("n c l -> c n l")
        xa = xp.tile([128, NPC * Lf], f32)
        nc.sync.dma_start(xa[:, 0:500].rearrange("c (n l) -> c n l", n=2),
                          xv[0:128, 0:2, :])
        nc.sync.dma_start(xa[:, 500:1000].rearrange("c (n l) -> c n l", n=2),
                          xv[0:128, 2:4, :])
        xb = xp.tile([128, NPC * Lf], f32)
        nc.sync.dma_start(xb[:, 0:500].rearrange("c (n l) -> c n l", n=2),
                          xv[128:256, 0:2, :])
        nc.sync.dma_start(xb[:, 500:1000].rearrange("c (n l) -> c n l", n=2),
                          xv[128:256, 2:4, :])
        xha = xp.tile([128, NPC * Lf], bf)
        xla = xp.tile([128, NPC * Lf], bf)
        xhb = xp.tile([128, NPC * Lf], bf)
        xlb = xp.tile([128, NPC * Lf], bf)
        for lo_, hi_ in [(0, 500), (500, 1000)]:
            nc.vector.tensor_copy(xha[:, lo_:hi_], xa[:, lo_:hi_])
            nc.vector.tensor_tensor(xla[:, lo_:hi_], xa[:, lo_:hi_], xha[:, lo_:hi_],
                                    AL.subtract)
            nc.vector.tensor_copy(xhb[:, lo_:hi_], xb[:, lo_:hi_])
            nc.vector.tensor_tensor(xlb[:, lo_:hi_], xb[:, lo_:hi_], xhb[:, lo_:hi_],
                                    AL.subtract)
        bo = const.tile([1, 1], f32)
        nc.scalar.dma_start(bo[:], bo_d[:])
        bm = const.tile([NH, 1], f32)
        nc.scalar.dma_start(bm[:], bm_d[:])
        rtu = const.tile([24, SEG], bf)
        nc.sync.dma_start(rtu[:], rtu_d[:])
        rmc = const.tile([3, NCB], bf)
        nc.sync.dma_start(rmc[:], rmc_d[:])
        m8 = const.tile([NH, 1], bf)
        nc.sync.dma_start(m8[:], m8_d[:])
        phi = const.tile([NPC, 1], f32)
        nc.sync.dma_start(phi[:], phi_d[:])

        # ---- PE clock warmup while x loads (results unused) ----
        wtile = const.tile([12, 512], bf)
        nc.vector.memset(wtile[:], 0.0)
        pwarm = pD.tile([128, 1024], f32, tag="s")
        for _ in range(6):
            nc.tensor.matmul(pwarm[:, 0:480], wtile[:, 0:128], wtile[:, 0:480],
                             start=True, stop=True)

        # ---- conv via bf16 hi/lo split: pc40 [40, 1000] ----
        NT = NPC * Lf  # 1000
        pc40 = pA.tile([40, 1024], f32, tag="s")
        for half, (c0, c1_) in enumerate([(0, 500), (500, 1000)]):
            pcol = half * 512
            out = pc40[:, pcol:pcol + 500]
            nc.tensor.matmul(out, wha[:], xha[:, c0:c1_], start=True, stop=False)
            nc.tensor.matmul(out, whb[:], xhb[:, c0:c1_], start=False, stop=False)
            nc.tensor.matmul(out, wla[:], xha[:, c0:c1_], start=False, stop=False)
            nc.tensor.matmul(out, wlb[:], xhb[:, c0:c1_], start=False, stop=False)
            nc.tensor.matmul(out, wha[:], xla[:, c0:c1_], start=False, stop=False)
            nc.tensor.matmul(out, whb[:], xlb[:, c0:c1_], start=False, stop=True)
        pcv = pc40[:].rearrange("p (b c) -> p b c", b=2)[:, :, 0:500]
        gall = frame.tile([1, NT], f32)
        nc.scalar.activation(gall[:].rearrange("p (b c) -> p b c", b=2),
                             pcv[32:33], AF.Exp, bias=bo[0:1, 0:1], scale=LN2)
        magbf = frame.tile([NH, NT], bf)
        nc.scalar.activation(magbf[:].rearrange("p (b c) -> p b c", b=2),
                             pcv[0:NH], AF.Exp, bias=bm[:, 0:1])

        # ---- g row -> gtp4 [4, 252] padded (pads via Pool copies) ----
        gtp4 = frame.tile([NPC, Lf + 2], f32)
        nc.sync.dma_start(gtp4[:, 1:Lf + 1],
                          gall[:].rearrange("p (n l) -> p n l", n=NPC))
        nc.vector.tensor_copy(gtp4[:, 0:1], gtp4[:, 1:2])
        nc.vector.tensor_copy(gtp4[:, Lf + 1:Lf + 2], gtp4[:, Lf:Lf + 1])

        # ---- g splits staged for DRAM hop: ghilo [4, 756] = [ghi, ghi, glo] ----
        ghilo = frame.tile([NPC, 756], bf)
        nc.gpsimd.tensor_copy(ghilo[:, 0:252], gtp4[:])
        nc.gpsimd.tensor_copy(ghilo[:, 252:504], gtp4[:])
        nc.gpsimd.tensor_tensor(ghilo[:, 504:756], gtp4[:], ghilo[:, 0:252], AL.subtract)
        nc.sync.dma_start(scr_g[:], ghilo[:])

        # ---- segment sums + scan (Pool, [4, 250]) ----
        gsum = frame.tile([NPC, Lf], f32)
        nc.vector.tensor_tensor(gsum[:], gtp4[:, 0:Lf], gtp4[:, 2:Lf + 2], AL.add)
        t2 = frame.tile([NPC, Lf], f32)
        nc.vector.tensor_scalar(t2[:], gtp4[:, 1:Lf + 1], c2, None, AL.mult)
        sst = frame.tile([NPC, Lf], f32)
        nc.vector.scalar_tensor_tensor(sst[:], gsum[:], c1, t2[:], AL.mult, AL.add)
        rS = frame.tile([NPC, Lf], f32)
        nc.vector.tensor_scalar(rS[:], sst[:], MAGIC, MAGIC, AL.add, AL.subtract)
        sf = frame.tile([NPC, Lf], f32)
        nc.vector.tensor_tensor(sf[:], sst[:], rS[:], AL.subtract)
        pinc = frame.tile([NPC, Lf], f32)
        nc.vector.tensor_tensor_scan(pinc[:], sf[:], sf[:], 0.0, AL.add, AL.bypass)
        base = frame.tile([NPC, Lf], f32)
        nc.vector.scalar_tensor_tensor(base[:], pinc[:], phi[:, 0:1], sf[:],
                                       AL.add, AL.subtract)
        rB = frame.tile([NPC, Lf], f32)
        nc.vector.tensor_scalar(rB[:], base[:], MAGIC, MAGIC, AL.add, AL.subtract)
        nc.vector.tensor_tensor(base[:], base[:], rB[:], AL.subtract)

        # ---- base splits staged: basell [4, 750] = [bhi, blo, MAGIC] ----
        basell = frame.tile([NPC, 500], bf)
        mrow8 = frame.tile([8, Lf], bf)
        nc.vector.memset(mrow8[:], MAGIC)
        nc.vector.tensor_copy(basell[:, 0:250], base[:])
        nc.vector.tensor_tensor(basell[:, 250:500], base[:], basell[:, 0:250],
                                AL.subtract)


        # ---- mbar = mean over harmonics (copies on Act) ----
        pmb = pB.tile([1, 1024], f32, tag="s")
        nc.tensor.matmul(pmb[:, 0:500], m8[:], magbf[:, 0:500], start=True, stop=True)
        nc.tensor.matmul(pmb[:, 512:1012], m8[:], magbf[:, 500:1000], start=True, stop=True)
        mb_all = frame.tile([1, NPC * (Lf + 2)], bf)   # padded per-sample [1, 4*252]
        mbv = mb_all[:].rearrange("p (n l) -> p n l", n=NPC)
        nc.vector.tensor_copy(mbv[:, 0:2, 1:Lf + 1],
                              pmb[:, 0:500].rearrange("p (n l) -> p n l", n=2))
        nc.vector.tensor_copy(mbv[:, 2:4, 1:Lf + 1],
                              pmb[:, 512:1012].rearrange("p (n l) -> p n l", n=2))
        nc.vector.tensor_copy(mbv[:, :, 0:1], mbv[:, :, 1:2])
        nc.vector.tensor_copy(mbv[:, :, Lf + 1:Lf + 2], mbv[:, :, Lf:Lf + 1])

        # ---- V12 [12, 1000] bf16, cols = (sample, segment) ----
        # rows 0-8: (copy c, tap tau) = 3c+tau; c0/c1 = ghi, c2 = glo
        # rows 9-11: bhi, blo, MAGIC
        V12 = vw.tile([24, NPC * Lf], bf)
        gview = scr_g[:].rearrange("n (c w) -> c n w", c=3)
        for tau, eng in [(0, nc.sync), (1, nc.scalar), (2, nc.gpsimd)]:
            eng.dma_start(
                V12[3 * tau:3 * tau + 3, :].rearrange("r (n l) -> r n l", n=NPC),
                gview[:, :, tau:tau + Lf])
        for tau, eng in [(0, nc.sync), (1, nc.scalar), (2, nc.gpsimd)]:
            eng.dma_start(
                V12[13 + 3 * tau:13 + 3 * tau + 3, :].rearrange("r (n l) -> r n l", n=NPC),
                gview[:, :, tau:tau + Lf])
        for r, row, eng in [(0, 9, nc.sync), (1, 10, nc.scalar), (0, 22, nc.gpsimd),
                            (1, 23, nc.sync)]:
            eng.dma_start(
                V12[row:row + 1, :].rearrange("p (n l) -> p n l", n=NPC),
                basell[:, 250 * r:250 * r + 250].rearrange("n (p l) -> n p l", p=1))
        nc.scalar.dma_start(V12[11:12, :].rearrange("p (n l) -> p n l", n=NPC),
                            mrow8[0:4, :].rearrange("n (p l) -> n p l", p=1))
        nc.gpsimd.dma_start(V12[12:13, :].rearrange("p (n l) -> p n l", n=NPC),
                            mrow8[4:8, :].rearrange("n (p l) -> n p l", p=1))

        W12 = vw.tile([3, NPC * Lf], bf)
        for tau, eng in [(0, nc.sync), (1, nc.gpsimd), (2, nc.scalar)]:
            eng.dma_start(W12[tau:tau + 1, :].rearrange("p (n l) -> p n l", n=NPC),
                          mbv[:, :, tau:tau + Lf])
        # ---- mag interp (coarse) for all 8 chunks into one psum tile ----
        pm = pC.tile([HP, 1024], f32, tag="s")
        for q in range(8):
            n_, h = q // 2, q % 2
            col = (q % 4) * NCB + (q // 4) * 512
            nc.tensor.matmul(pm[:, col:col + NCB],
                             W12[:, n_ * Lf + h * HP:n_ * Lf + (h + 1) * HP],
                             rmc[:], start=True, stop=True)
        mvs = frame.tile([HP, 8 * NCB], f32)
        nc.vector.tensor_copy(mvs[:].rearrange("p (b c) -> p b c", b=2, c=4 * NCB),
                              pm[:].rearrange("p (b c) -> p b c", b=2)[:, :, 0:4 * NCB])

        # ---- chunk loop ----
        upools = [pA, pB, pC, pD]
        waves = {}
        for q in range(8):
            n_, h = q // 2, q % 2
            cols = slice(n_ * Lf + h * HP, n_ * Lf + (h + 1) * HP)
            pv = upools[q % 4].tile([HP, 1024], f32, tag="s")
            nc.tensor.matmul(pv[:, 0:480], V12[:, cols],
                             rtu[:, 0:480], start=True, stop=True)
            nc.tensor.matmul(pv[:, 512:992], V12[:, cols],
                             rtu[:, 480:960], start=True, stop=True)
            # psum already holds v = u - round(u); sn = sin(2pi*v) = sin(2pi*u)
            snb = sn_p.tile([HP, SEG], bf)
            nc.scalar.activation(snb[:].rearrange("p (b c) -> p b c", b=2),
                                 pv[:].rearrange("p (b c) -> p b c", b=2)[:, :, 0:480],
                                 AF.Sin, scale=TWO_PI * (1.0 - 2e-6))
            # wave = sn * mag  (Pool; mag piecewise-constant over CB samples)
            wave = wv_p.tile([HP, SEG], bf)
            mcol = (q % 4) * NCB + (q // 4) * 4 * NCB
            nc.gpsimd.tensor_tensor(
                wave[:].rearrange("p (i j) -> p i j", j=CB),
                snb[:].rearrange("p (i j) -> p i j", j=CB),
                mvs[:, mcol:mcol + NCB].rearrange("p (i j) -> p i j", j=1).broadcast_to(
                    (HP, NCB, CB)),
                AL.mult)
            nc.sync.dma_start(out_d[n_, h * HP:(h + 1) * HP, :], wave[:])

    nc.compile()
    return nc


def kernel(x, phi, w_mag, b_mag, w_oct, b_oct):
    from concourse.bass_utils import run_bass_kernel_spmd
    import ml_dtypes

    if "nc" not in _cache:
        _cache["nc"] = _build()
    nc = _cache["nc"]

    import ml_dtypes
    rtu, rmc, m8 = _consts()
    wT40 = np.zeros((C, 40), np.float32)
    wT40[:, 0:NH] = w_mag[:, :, 0].T
    wT40[:, 32] = w_oct[0, :, 0]
    wh = wT40.astype(ml_dtypes.bfloat16)
    wl = (wT40 - wh.astype(np.float32)).astype(ml_dtypes.bfloat16)
    bm = np.asarray(b_mag, np.float32).reshape(NH, 1)
    bo = np.array([[np.log(220.0) + np.log(2.0) * float(np.asarray(b_oct).ravel()[0])]],
                  np.float32)
    in_maps = []
    for c in range(NCORES):
        in_maps.append(dict(
            x=np.ascontiguousarray(x[c * NPC:(c + 1) * NPC]).astype(np.float32),
            phi=np.ascontiguousarray(phi[c * NPC:(c + 1) * NPC, 0]).astype(np.float32),
            wh=wh, wl=wl, bmag=bm, boct=bo, rtu=rtu, rmc=rmc, m8=m8,
        ))
    res = run_bass_kernel_spmd(nc, in_maps, core_ids=list(range(NCORES)))
    waves = [res.results[c]["wave"].astype(np.float32).reshape(NPC, 1, Lw)
             for c in range(NCORES)]
    return np.concatenate(waves, axis=0)


# revision 8
# speedup vs baseline: 1.0352x; 1.0321x over previous
import numpy as np

SR, SEG, NH, BASE_F = 48000, 960, 8, 220.0
N, C, Lf = 32, 256, 250
Lw = Lf * SEG
NCORES = 8
NPC = N // NCORES   # 4 samples per core
HP = Lf // 2        # 125 segments per half
MAGIC = 12582912.0  # 1.5*2^23 (exactly representable in bf16)
CB = 8              # coarse mag block (samples per constant-mag block)
NCB = SEG // CB     # 120 coarse blocks per segment

_cache = {}


def _bf16(x):
    import ml_dtypes
    return np.asarray(x, np.float32).astype(ml_dtypes.bfloat16)


def _consts():
    import ml_dtypes
    s = np.arange(SEG, dtype=np.float64)
    delta = (s + 0.5) / SEG - 0.5
    lo = s < SEG // 2
    a_s = np.where(lo, -delta, 0.0)
    b_s = np.where(lo, 1 + delta, 1 - delta)
    d_s = np.where(lo, 0.0, delta)
    A = np.cumsum(a_s) / SR
    B = np.cumsum(b_s) / SR
    D = np.cumsum(d_s) / SR
    Cm = np.stack([A, B, D]).astype(np.float32)          # [3, 960]
    C_hi = Cm.astype(ml_dtypes.bfloat16)
    C_lo = (Cm - C_hi.astype(np.float32)).astype(ml_dtypes.bfloat16)
    one = np.ones(SEG, np.float32)
    # V rows: [gtaps(9), bhi, blo, MAGIC, MAGIC, gtaps(9), bhi, blo]
    # rt rows: [-C..., -1, -1, +1, -1, +C..., +1, +1] => psum = u - round(u)
    crows = []
    for t in range(3):
        crows += [C_hi[t].astype(np.float32), C_lo[t].astype(np.float32),
                  C_hi[t].astype(np.float32)]
    neg = [-r for r in crows] + [-one, -one]
    mid = [one, -one]
    pos = crows + [one, one]
    rtu = np.stack(neg + mid + pos).astype(ml_dtypes.bfloat16)  # [24, 960]
    # mag interp weights at coarse block centers (piecewise-constant blocks)
    jj = np.arange(NCB) * CB + (CB - 1) / 2.0
    j0 = np.floor(jj).astype(int)
    w = (jj - j0).astype(np.float64)
    rmc = np.stack([
        (1 - w) * np.stack([a_s, b_s, d_s])[:, j0][i] + w * np.stack([a_s, b_s, d_s])[:, np.minimum(j0 + 1, SEG - 1)][i]
        for i in range(3)
    ]).astype(ml_dtypes.bfloat16)                        # [3, 120]
    m8 = np.full((NH, 1), 1.0 / NH, ml_dtypes.bfloat16)
    return rtu, rmc, m8


def _build():
    import concourse.bacc as bacc
    import concourse.mybir as mybir
    import concourse.tile as tile
    from contextlib import ExitStack

    f32 = mybir.dt.float32
    bf = mybir.dt.bfloat16
    AF = mybir.ActivationFunctionType
    AL = mybir.AluOpType
    LN2 = float(np.log(2.0))
    TWO_PI = float(2.0 * np.pi)
    c1, c2 = 120.0 / SR, 720.0 / SR

    nc = bacc.Bacc("TRN2", target_bir_lowering=False, debug=False)
    x_d = nc.dram_tensor("x", [NPC, C, Lf], f32, kind="ExternalInput")
    phi_d = nc.dram_tensor("phi", [NPC, 1], f32, kind="ExternalInput")
    wh_d = nc.dram_tensor("wh", [C, 40], bf, kind="ExternalInput")
    wl_d = nc.dram_tensor("wl", [C, 40], bf, kind="ExternalInput")
    bm_d = nc.dram_tensor("bmag", [NH, 1], f32, kind="ExternalInput")
    bo_d = nc.dram_tensor("boct", [1, 1], f32, kind="ExternalInput")
    rtu_d = nc.dram_tensor("rtu", [24, SEG], bf, kind="ExternalInput")
    rmc_d = nc.dram_tensor("rmc", [3, NCB], bf, kind="ExternalInput")
    m8_d = nc.dram_tensor("m8", [NH, 1], bf, kind="ExternalInput")
    out_d = nc.dram_tensor("wave", [NPC, Lf, SEG], bf, kind="ExternalOutput")
    scr_g = nc.dram_tensor("scr_g", [NPC, 756], bf, kind="Internal")
    scr_b = nc.dram_tensor("scr_b", [NPC, 500], bf, kind="Internal")


    with tile.TileContext(nc) as tc, ExitStack() as ctx:
        const = ctx.enter_context(tc.tile_pool(name="const", bufs=1))
        xp = ctx.enter_context(tc.tile_pool(name="xp", bufs=1))
        frame = ctx.enter_context(tc.tile_pool(name="frame", bufs=1))
        vw = ctx.enter_context(tc.tile_pool(name="vw", bufs=1))
        vt_p = ctx.enter_context(tc.tile_pool(name="vt_p", bufs=2))
        sn_p = ctx.enter_context(tc.tile_pool(name="sn_p", bufs=2))
        wv_p = ctx.enter_context(tc.tile_pool(name="wv_p", bufs=3))
        # PSUM: four 2-bank pools, tiles rotate through them
        pA = ctx.enter_context(tc.tile_pool(name="pA", bufs=1, space="PSUM"))
        pB = ctx.enter_context(tc.tile_pool(name="pB", bufs=1, space="PSUM"))
        pC = ctx.enter_context(tc.tile_pool(name="pC", bufs=1, space="PSUM"))
        pD = ctx.enter_context(tc.tile_pool(name="pD", bufs=1, space="PSUM"))

        # ---- weights first (scalar q), x by halves on both queues ----
        wha = const.tile([128, 40], bf)
        nc.scalar.dma_start(wha[:], wh_d[0:128, :])
        whb = const.tile([128, 40], bf)
        nc.scalar.dma_start(whb[:], wh_d[128:256, :])
        wla = const.tile([128, 40], bf)
        nc.scalar.dma_start(wla[:], wl_d[0:128, :])
        wlb = const.tile([128, 40], bf)
        nc.scalar.dma_start(wlb[:], wl_d[128:256, :])
        xv = x_d[:].rearrange("n c l -> c n l")
        xa = xp.tile([128, NPC * Lf], f32)
        nc.sync.dma_start(xa[:, 0:500].rearrange("c (n l) -> c n l", n=2),
                          xv[0:128, 0:2, :])
        nc.sync.dma_start(xa[:, 500:1000].rearrange("c (n l) -> c n l", n=2),
                          xv[0:128, 2:4, :])
        xb = xp.tile([128, NPC * Lf], f32)
        nc.sync.dma_start(xb[:, 0:500].rearrange("c (n l) -> c n l", n=2),
                          xv[128:256, 0:2, :])
        nc.sync.dma_start(xb[:, 500:1000].rearrange("c (n l) -> c n l", n=2),
                          xv[128:256, 2:4, :])
        xha = xp.tile([128, NPC * Lf], bf)
        xla = xp.tile([128, NPC * Lf], bf)
        xhb = xp.tile([128, NPC * Lf], bf)
        xlb = xp.tile([128, NPC * Lf], bf)
        for lo_, hi_ in [(0, 500), (500, 1000)]:
            nc.vector.tensor_copy(xha[:, lo_:hi_], xa[:, lo_:hi_])
            nc.vector.tensor_tensor(xla[:, lo_:hi_], xa[:, lo_:hi_], xha[:, lo_:hi_],
                                    AL.subtract)
            nc.vector.tensor_copy(xhb[:, lo_:hi_], xb[:, lo_:hi_])
            nc.vector.tensor_tensor(xlb[:, lo_:hi_], xb[:, lo_:hi_], xhb[:, lo_:hi_],
                                    AL.subtract)
        bo = const.tile([1, 1], f32)
        nc.scalar.dma_start(bo[:], bo_d[:])
        bm = const.tile([NH, 1], f32)
        nc.scalar.dma_start(bm[:], bm_d[:])
        rtu = const.tile([24, SEG], bf)
        nc.sync.dma_start(rtu[:], rtu_d[:])
        rmc = const.tile([3, NCB], bf)
        nc.sync.dma_start(rmc[:], rmc_d[:])
        m8 = const.tile([NH, 1], bf)
        nc.sync.dma_start(m8[:], m8_d[:])
        phi = const.tile([NPC, 1], f32)
        nc.sync.dma_start(phi[:], phi_d[:])

        # ---- PE clock warmup while x loads (results unused) ----
        wtile = const.tile([12, 512], bf)
        nc.vector.memset(wtile[:], 0.0)
        pwarm = pD.tile([128, 1024], f32, tag="s")
        for _ in range(6):
            nc.tensor.matmul(pwarm[:, 0:480], wtile[:, 0:128], wtile[:, 0:480],
                             start=True, stop=True)

        # ---- conv via bf16 hi/lo split: pc40 [40, 1000] ----
        NT = NPC * Lf  # 1000
        pc40 = pA.tile([40, 1024], f32, tag="s")
        for half, (c0, c1_) in enumerate([(0, 500), (500, 1000)]):
            pcol = half * 512
            out = pc40[:, pcol:pcol + 500]
            nc.tensor.matmul(out, wha[:], xha[:, c0:c1_], start=True, stop=False)
            nc.tensor.matmul(out, whb[:], xhb[:, c0:c1_], start=False, stop=False)
            nc.tensor.matmul(out, wla[:], xha[:, c0:c1_], start=False, stop=False)
            nc.tensor.matmul(out, wlb[:], xhb[:, c0:c1_], start=False, stop=False)
            nc.tensor.matmul(out, wha[:], xla[:, c0:c1_], start=False, stop=False)
            nc.tensor.matmul(out, whb[:], xlb[:, c0:c1_], start=False, stop=True)
        pcv = pc40[:].rearrange("p (b c) -> p b c", b=2)[:, :, 0:500]
        gall = frame.tile([1, NT], f32)
        nc.scalar.activation(gall[:].rearrange("p (b c) -> p b c", b=2),
                             pcv[32:33], AF.Exp, bias=bo[0:1, 0:1], scale=LN2)
        magbf = frame.tile([NH, NT], bf)
        nc.scalar.activation(magbf[:].rearrange("p (b c) -> p b c", b=2),
                             pcv[0:NH], AF.Exp, bias=bm[:, 0:1])

        # ---- g row -> gtp4 [4, 252] padded (pads via Pool copies) ----
        gtp4 = frame.tile([NPC, Lf + 2], f32)
        nc.sync.dma_start(gtp4[:, 1:Lf + 1],
                          gall[:].rearrange("p (n l) -> p n l", n=NPC))
        nc.vector.tensor_copy(gtp4[:, 0:1], gtp4[:, 1:2])
        nc.vector.tensor_copy(gtp4[:, Lf + 1:Lf + 2], gtp4[:, Lf:Lf + 1])

        # ---- g splits staged for DRAM hop: ghilo [4, 756] = [ghi, ghi, glo] ----
        ghilo = frame.tile([NPC, 756], bf)
        nc.gpsimd.tensor_copy(ghilo[:, 0:252], gtp4[:])
        nc.gpsimd.tensor_copy(ghilo[:, 252:504], gtp4[:])
        nc.gpsimd.tensor_tensor(ghilo[:, 504:756], gtp4[:], ghilo[:, 0:252], AL.subtract)
        nc.sync.dma_start(scr_g[:], ghilo[:])

        # ---- segment sums + scan (Pool, [4, 250]) ----
        gsum = frame.tile([NPC, Lf], f32)
        nc.vector.tensor_tensor(gsum[:], gtp4[:, 0:Lf], gtp4[:, 2:Lf + 2], AL.add)
        t2 = frame.tile([NPC, Lf], f32)
        nc.vector.tensor_scalar(t2[:], gtp4[:, 1:Lf + 1], c2, None, AL.mult)
        sst = frame.tile([NPC, Lf], f32)
        nc.vector.scalar_tensor_tensor(sst[:], gsum[:], c1, t2[:], AL.mult, AL.add)
        rS = frame.tile([NPC, Lf], f32)
        nc.vector.tensor_scalar(rS[:], sst[:], MAGIC, MAGIC, AL.add, AL.subtract)
        sf = frame.tile([NPC, Lf], f32)
        nc.vector.tensor_tensor(sf[:], sst[:], rS[:], AL.subtract)
        pinc = frame.tile([NPC, Lf], f32)
        nc.vector.tensor_tensor_scan(pinc[:], sf[:], sf[:], 0.0, AL.add, AL.bypass)
        base = frame.tile([NPC, Lf], f32)
        nc.vector.scalar_tensor_tensor(base[:], pinc[:], phi[:, 0:1], sf[:],
                                       AL.add, AL.subtract)
        rB = frame.tile([NPC, Lf], f32)
        nc.vector.tensor_scalar(rB[:], base[:], MAGIC, MAGIC, AL.add, AL.subtract)
        nc.vector.tensor_tensor(base[:], base[:], rB[:], AL.subtract)

        # ---- base splits staged: basell [4, 750] = [bhi, blo, MAGIC] ----
        basell = frame.tile([NPC, 500], bf)
        mrow8 = frame.tile([8, Lf], bf)
        nc.vector.memset(mrow8[:], MAGIC)
        nc.vector.tensor_copy(basell[:, 0:250], base[:])
        nc.vector.tensor_tensor(basell[:, 250:500], base[:], basell[:, 0:250],
                                AL.subtract)
        nc.sync.dma_start(scr_b[:], basell[:])


        # ---- mbar = mean over harmonics (copies on Act) ----
        pmb = pB.tile([1, 1024], f32, tag="s")
        nc.tensor.matmul(pmb[:, 0:500], m8[:], magbf[:, 0:500], start=True, stop=True)
        nc.tensor.matmul(pmb[:, 512:1012], m8[:], magbf[:, 500:1000], start=True, stop=True)
        mb_all = frame.tile([1, NPC * (Lf + 2)], bf)   # padded per-sample [1, 4*252]
        mbv = mb_all[:].rearrange("p (n l) -> p n l", n=NPC)
        nc.vector.tensor_copy(mbv[:, 0:2, 1:Lf + 1],
                              pmb[:, 0:500].rearrange("p (n l) -> p n l", n=2))
        nc.vector.tensor_copy(mbv[:, 2:4, 1:Lf + 1],
                              pmb[:, 512:1012].rearrange("p (n l) -> p n l", n=2))
        nc.vector.tensor_copy(mbv[:, :, 0:1], mbv[:, :, 1:2])
        nc.vector.tensor_copy(mbv[:, :, Lf + 1:Lf + 2], mbv[:, :, Lf:Lf + 1])

        # ---- V12 [12, 1000] bf16, cols = (sample, segment) ----
        # rows 0-8: (copy c, tap tau) = 3c+tau; c0/c1 = ghi, c2 = glo
        # rows 9-11: bhi, blo, MAGIC
        V12 = vw.tile([24, NPC * Lf], bf)
        gview = scr_g[:].rearrange("n (c w) -> c n w", c=3)
        for tau, eng in [(0, nc.sync), (1, nc.scalar), (2, nc.gpsimd)]:
            eng.dma_start(
                V12[3 * tau:3 * tau + 3, :].rearrange("r (n l) -> r n l", n=NPC),
                gview[:, :, tau:tau + Lf])
        for tau, eng in [(0, nc.sync), (1, nc.scalar), (2, nc.gpsimd)]:
            eng.dma_start(
                V12[13 + 3 * tau:13 + 3 * tau + 3, :].rearrange("r (n l) -> r n l", n=NPC),
                gview[:, :, tau:tau + Lf])
        bview = scr_b[:].rearrange("n (r l) -> r n l", r=2)
        nc.sync.dma_start(V12[9:11, :].rearrange("r (n l) -> r n l", n=NPC), bview)
        nc.scalar.dma_start(V12[22:24, :].rearrange("r (n l) -> r n l", n=NPC), bview)
        nc.scalar.dma_start(V12[11:12, :].rearrange("p (n l) -> p n l", n=NPC),
                            mrow8[0:4, :].rearrange("n (p l) -> n p l", p=1))
        nc.gpsimd.dma_start(V12[12:13, :].rearrange("p (n l) -> p n l", n=NPC),
                            mrow8[4:8, :].rearrange("n (p l) -> n p l", p=1))

        W12 = vw.tile([3, NPC * Lf], bf)
        for tau, eng in [(0, nc.sync), (1, nc.gpsimd), (2, nc.scalar)]:
            eng.dma_start(W12[tau:tau + 1, :].rearrange("p (n l) -> p n l", n=NPC),
                          mbv[:, :, tau:tau + Lf])
        # ---- mag interp (coarse) for all 8 chunks into one psum tile ----
        pm = pC.tile([HP, 1024], f32, tag="s")
        for q in range(8):
            n_, h = q // 2, q % 2
            col = (q % 4) * NCB + (q // 4) * 512
            nc.tensor.matmul(pm[:, col:col + NCB],
                             W12[:, n_ * Lf + h * HP:n_ * Lf + (h + 1) * HP],
                             rmc[:], start=True, stop=True)
        mvs = frame.tile([HP, 8 * NCB], f32)
        nc.vector.tensor_copy(mvs[:].rearrange("p (b c) -> p b c", b=2, c=4 * NCB),
                              pm[:].rearrange("p (b c) -> p b c", b=2)[:, :, 0:4 * NCB])

        # ---- chunk loop ----
        upools = [pA, pB, pC, pD]
        waves = {}
        for q in range(8):
            n_, h = q // 2, q % 2
            cols = slice(n_ * Lf + h * HP, n_ * Lf + (h + 1) * HP)
            pv = upools[q % 4].tile([HP, 1024], f32, tag="s")
            nc.tensor.matmul(pv[:, 0:480], V12[:, cols],
                             rtu[:, 0:480], start=True, stop=True)
            nc.tensor.matmul(pv[:, 512:992], V12[:, cols],
                             rtu[:, 480:960], start=True, stop=True)
            # psum already holds v = u - round(u); sn = sin(2pi*v) = sin(2pi*u)
            snb = sn_p.tile([HP, SEG], bf)
            nc.scalar.activation(snb[:].rearrange("p (b c) -> p b c", b=2),
                                 pv[:].rearrange("p (b c) -> p b c", b=2)[:, :, 0:480],
                                 AF.Sin, scale=TWO_PI * (1.0 - 2e-6))
            # wave = sn * mag  (Pool; mag piecewise-constant over CB samples)
            wave = wv_p.tile([HP, SEG], bf)
            mcol = (q % 4) * NCB + (q // 4) * 4 * NCB
            nc.gpsimd.tensor_tensor(
                wave[:].rearrange("p (i j) -> p i j", j=CB),
                snb[:].rearrange("p (i j) -> p i j", j=CB),
                mvs[:, mcol:mcol + NCB].rearrange("p (i j) -> p i j", j=1).broadcast_to(
                    (HP, NCB, CB)),
                AL.mult)
            nc.sync.dma_start(out_d[n_, h * HP:(h + 1) * HP, :], wave[:])

    nc.compile()
    return nc


def kernel(x, phi, w_mag, b_mag, w_oct, b_oct):
    from concourse.bass_utils import run_bass_kernel_spmd
    import ml_dtypes

    if "nc" not in _cache:
        _cache["nc"] = _build()
    nc = _cache["nc"]

    import ml_dtypes
    rtu, rmc, m8 = _consts()
    wT40 = np.zeros((C, 40), np.float32)
    wT40[:, 0:NH] = w_mag[:, :, 0].T
    wT40[:, 32] = w_oct[0, :, 0]
    wh = wT40.astype(ml_dtypes.bfloat16)
    wl = (wT40 - wh.astype(np.float32)).astype(ml_dtypes.bfloat16)
    bm = np.asarray(b_mag, np.float32).reshape(NH, 1)
    bo = np.array([[np.log(220.0) + np.log(2.0) * float(np.asarray(b_oct).ravel()[0])]],
                  np.float32)
    in_maps = []
    for c in range(NCORES):
        in_maps.append(dict(
            x=np.ascontiguousarray(x[c * NPC:(c + 1) * NPC]).astype(np.float32),
            phi=np.ascontiguousarray(phi[c * NPC:(c + 1) * NPC, 0]).astype(np.float32),
            wh=wh, wl=wl, bmag=bm, boct=bo, rtu=rtu, rmc=rmc, m8=m8,
        ))
    res = run_bass_kernel_spmd(nc, in_maps, core_ids=list(range(NCORES)))
    waves = [res.results[c]["wave"].astype(np.float32).reshape(NPC, 1, Lw)
             for c in range(NCORES)]
    return np.concatenate(waves, axis=0)
